# revision 15
# baseline (speedup 1.0000x reference)
"""YOLOv3-style detector head (decode + global top-K + per-image NMS) on 8
Trainium2 NeuronCores via Bass/Tile.

Batch B=32 is sharded 4 images/core over 8 cores (data-parallel), per the
problem's sharding hint. Two SPMD launches:

  Launch 1 (device): stream the objectness planes of the 4 images into a
    [128, 712] layout (32 partitions/image) and emit the per-partition top-8
    candidate indices (vector MAX8 / FIND_INDEX8).
  Host: rank the 256 candidates per image by their exact f32 logits (pure
    post-processing of device output), keep the top R=64, and gather the
    payloads (tx/ty/tw/th, class logits, grid/anchor constants) at the
    device-chosen indices -- indexed gather only, no arithmetic.
  Launch 2 (device): sigmoid/exp box decode, pairwise IoU adjacency,
    depth-1 greedy-NMS keep flags, 80-class argmax, masked output rows.
    Layout: 128 partitions = 2 images x 64 slots, 2 free-dim image blocks,
    so every elementwise op covers 2 images at once and the j-side geometry
    broadcast is a single K=2 matmul against a block-selector.
  Host: merge the 32 per-image candidate lists into the [1024, 7] output
    ordered by (score desc, reference index asc), zeroing suppressed rows.

Selection is done on raw objectness logits (monotone in sigmoid), so ordering
and argmax are exact input-value comparisons; sigmoid/exp only affect emitted
values, never which boxes are chosen.
"""

import os
import numpy as np
from contextlib import ExitStack

import concourse.bass as bass
import concourse.tile as tile
import concourse.mybir as mybir
from concourse import bacc
from concourse.bass_utils import run_bass_kernel_spmd

# ---------------------------------------------------------------- constants
B = 32
N_CORES = 8
IPC = B // N_CORES          # images per core
K_OUT = 1024
NMS_IOU = 0.3
IOU_C = float(NMS_IOU / (1.0 + NMS_IOU))   # inter > IOU_C*(a_i+a_j)
GRIDS = [19, 38, 76]
STRIDES = [32.0, 16.0, 8.0]
ANCHORS_NAME = ["anchors_13", "anchors_26", "anchors_52"]
OUT_NAME = ["output_13", "output_26", "output_52"]
PPART = 712                 # boxes per partition: 4 images x 32 partitions
NPAD = 32 * PPART           # padded boxes per image (22784)
R = 64                      # candidate slots per image (rank-trimmed)
BIG = 1.0e30
NEG = -1.0e30
_f32 = mybir.dt.float32


def _tables():
    # flat my-order stream: scale-major, anchor, cell; padded tail
    gx, gy, st, ref, s_l, a_l, c_l = [], [], [], [], [], [], []
    ref_off = [0, 3 * GRIDS[0] ** 2, 3 * (GRIDS[0] ** 2 + GRIDS[1] ** 2)]
    for s, g in enumerate(GRIDS):
        c = np.arange(g * g)
        for a in range(3):
            gx.append(c % g)
            gy.append(c // g)
            st.append(np.full(g * g, STRIDES[s]))
            ref.append(ref_off[s] + c * 3 + a)
            s_l.append(np.full(g * g, s))
            a_l.append(np.full(g * g, a))
            c_l.append(c)
    def cat(parts, pad, dt):
        x = np.concatenate(parts).astype(dt)
        return np.concatenate([x, np.full(NPAD - len(x), pad, dt)])
    sa = np.stack([cat(s_l, 0, np.int64), cat(a_l, 0, np.int64)], axis=1)
    return (cat(gx, 0, np.float32), cat(gy, 0, np.float32),
            cat(st, 0, np.float32), cat(ref, -1, np.int64), sa,
            cat(c_l, 0, np.int64))


GXC, GYC, STC, REFC, SAC, CELLC = _tables()
_SCALE_GLOBAL_OFF = [0, B * 3 * GRIDS[0] ** 2,
                     B * 3 * (GRIDS[0] ** 2 + GRIDS[1] ** 2)]

# =================================================================== L1
_l1_cache = {}


def _build_l1():
    if "nc" in _l1_cache:
        return _l1_cache["nc"]
    nc = bacc.Bacc("TRN2", target_bir_lowering=False, debug=False)
    x_d = nc.dram_tensor("conf", [128, PPART], _f32, kind="ExternalInput")
    i_d = nc.dram_tensor("mi", [128, 8], mybir.dt.uint16, kind="ExternalOutput")
    with ExitStack() as ctx:
        tc = ctx.enter_context(tile.TileContext(nc))
        pool = ctx.enter_context(tc.tile_pool(name="p", bufs=1))
        k = pool.tile([128, PPART], _f32)
        xap = x_d.ap()
        nc.sync.dma_start(k[0:64, :], xap[0:64, :])
        nc.scalar.dma_start(k[64:128, :], xap[64:128, :])
        m1 = pool.tile([128, 8], _f32)
        mi = pool.tile([128, 8], mybir.dt.uint16)
        nc.vector.max(out=m1[:], in_=k[:])
        nc.vector.max_index(out=mi[:], in_max=m1[:], in_values=k[:])
        nc.sync.dma_start(i_d.ap(), mi[:])
    nc.compile()
    _l1_cache["nc"] = nc
    return nc


def _l1_pack(inputs):
    """Per-core packed conf planes; also returned per-image flat for host use."""
    packs, flats = [], []
    for core in range(N_CORES):
        k = np.full((IPC, NPAD), NEG, np.float32)
        for b in range(IPC):
            img = core * IPC + b
            parts = [inputs[OUT_NAME[s]][img, a * 85 + 4].reshape(-1)
                     for s in range(3) for a in range(3)]
            flat = np.concatenate(parts)
            k[b, :flat.size] = flat
        packs.append({"conf": k.reshape(128, PPART)})
        flats.append(k)
    return packs, flats


# =================================================================== L2
_l2_cache = {}

# CT tile columns: [j-sig 0:128 | key 128:130 | tx 130:132 | ty 132:134 |
#                   tw 134:136 | th 136:138 | j-exp 138:266]
CT_W = 266
# GI columns: gx(0:2) gy(2:4) st(4:6) st2(6:8) aw(8:10) ah(10:12) thr(12:14)
GI_W = 14
# JG columns: jgx(0:64) jgy(64:128) jst(128:192) jst2(192:256) jaw(256:320)
#             jah(320:384)
JG_W = 384


def _build_l2():
    if "nc" in _l2_cache:
        return _l2_cache["nc"]
    nc = bacc.Bacc("TRN2", target_bir_lowering=False, debug=False)
    ct_d = nc.dram_tensor("ct", [128, CT_W], _f32, kind="ExternalInput")
    gi_d = nc.dram_tensor("gi", [128, GI_W], _f32, kind="ExternalInput")
    jg_d = nc.dram_tensor("jg", [34, JG_W], _f32, kind="ExternalInput")
    sel_d = nc.dram_tensor("sel", [34, 128], _f32, kind="ExternalInput")
    tri_d = nc.dram_tensor("tri", [128, R], _f32, kind="ExternalInput")
    cls_d = nc.dram_tensor("cls", [128, 2 * 80], _f32, kind="ExternalInput")
    out_d = nc.dram_tensor("out", [128, 16], _f32, kind="ExternalOutput")

    Act = mybir.ActivationFunctionType
    Alu = mybir.AluOpType
    with ExitStack() as ctx:
        tc = ctx.enter_context(tile.TileContext(nc))
        pool = ctx.enter_context(tc.tile_pool(name="p", bufs=1))
        ppool = ctx.enter_context(tc.tile_pool(name="ps", bufs=1, space="PSUM"))

        # input DMAs all on the sync queue, ordered by first use; CT is
        # split into sig/exp halves so the sigmoid waits only on the first
        ctap = ct_d.ap()
        CTA = pool.tile([128, 134], _f32)
        nc.sync.dma_start(CTA[:], ctap[:, 0:134])
        CTB = pool.tile([128, 132], _f32)
        nc.sync.dma_start(CTB[:], ctap[:, 134:266])
        JG = pool.tile([34, JG_W], _f32)
        nc.sync.dma_start(JG[:], jg_d.ap())
        GI = pool.tile([128, GI_W], _f32)
        nc.sync.dma_start(GI[:], gi_d.ap())
        CLS = pool.tile([128, 160], _f32)
        nc.sync.dma_start(CLS[:], cls_d.ap())
        SEL = pool.tile([34, 128], _f32)
        nc.sync.dma_start(SEL[:], sel_d.ap())
        TRI = pool.tile([128, R], _f32)
        nc.sync.dma_start(TRI[:], tri_d.ap())

        # sigmoid over the sig fields only; exp table load overlaps decode
        S = pool.tile([128, 134], _f32)
        nc.scalar.activation(S[:], CTA[:], Act.Sigmoid)
        EX = pool.tile([128, 132], _f32)
        nc.scalar.activation(EX[:], CTB[:], Act.Exp)

        # ---- j-side decode into J2 = [x1 y1 | x2 y2 | c*area] ---------
        JXY = pool.tile([34, 128], _f32)
        nc.vector.tensor_tensor(out=JXY[:], in0=S[0:34, 0:128],
                                in1=JG[:, 0:128], op=Alu.add)
        nc.vector.tensor_tensor(out=JXY[:], in0=JXY[:], in1=JG[:, 128:256],
                                op=Alu.mult)
        JWH = pool.tile([34, 128], _f32)
        nc.vector.tensor_tensor(out=JWH[:], in0=EX[0:34, 4:132],
                                in1=JG[:, 256:384], op=Alu.mult)
        J2 = pool.tile([34, 320], _f32)
        nc.vector.scalar_tensor_tensor(J2[:, 0:128], JWH[:], -0.5, JXY[:],
                                       op0=Alu.mult, op1=Alu.add)
        nc.vector.scalar_tensor_tensor(J2[:, 128:256], JWH[:], 0.5, JXY[:],
                                       op0=Alu.mult, op1=Alu.add)
        nc.vector.scalar_tensor_tensor(J2[:, 256:320], JWH[:, 0:64], IOU_C,
                                       JWH[:, 64:128], op0=Alu.mult,
                                       op1=Alu.mult)

        # ---- i-side decode: DEC = [cx(2) cy(2) w(2) h(2)] -------------
        DEC = pool.tile([128, 8], _f32)
        CXY = DEC[:, 0:4]
        WH = DEC[:, 4:8]
        nc.vector.tensor_tensor(out=CXY, in0=S[:, 130:134], in1=GI[:, 0:4],
                                op=Alu.add)
        nc.vector.tensor_tensor(out=CXY, in0=CXY, in1=GI[:, 4:8],
                                op=Alu.mult)
        nc.vector.tensor_tensor(out=WH, in0=EX[:, 0:4], in1=GI[:, 8:12],
                                op=Alu.mult)
        C1 = pool.tile([128, 4], _f32)   # x1(2) y1(2)
        C2 = pool.tile([128, 4], _f32)   # x2(2) y2(2)
        nc.vector.scalar_tensor_tensor(C1[:], WH, -0.5, CXY,
                                       op0=Alu.mult, op1=Alu.add)
        nc.vector.scalar_tensor_tensor(C2[:], WH, 0.5, CXY,
                                       op0=Alu.mult, op1=Alu.add)
        ARC = pool.tile([128, 2], _f32)  # IOU_C * area_i
        nc.vector.scalar_tensor_tensor(ARC[:], DEC[:, 4:6], IOU_C,
                                       DEC[:, 6:8], op0=Alu.mult,
                                       op1=Alu.mult)
        PASS = pool.tile([128, 2], _f32)
        nc.vector.tensor_tensor(out=PASS[:], in0=CTA[:, 128:130],
                                in1=GI[:, 12:14], op=Alu.is_gt)

        # ---- broadcast j-geometry to all partitions (one matmul/pb) ---
        ps_bc = [ppool.tile([128, 320], _f32, tag=f"bc{pb}", name=f"bc{pb}")
                 for pb in range(2)]
        for pb in range(2):
            nc.tensor.matmul(ps_bc[pb][:], SEL[32 * pb:32 * pb + 2, :],
                             J2[32 * pb:32 * pb + 2, :])

        KEEP = pool.tile([128, 2], _f32)
        OUT = pool.tile([128, 16], _f32)
        for pb in range(2):
            BC = ps_bc[pb]
            # ---- IoU adjacency ---------------------------------------
            T1 = pool.tile([128, R], _f32)
            T2 = pool.tile([128, R], _f32)
            U1 = pool.tile([128, R], _f32)
            U2 = pool.tile([128, R], _f32)
            nc.vector.tensor_scalar(out=T1[:], in0=BC[:, 0:64],
                                    scalar1=C1[:, pb:pb + 1], scalar2=None,
                                    op0=Alu.max)
            nc.vector.tensor_scalar(out=T2[:], in0=BC[:, 64:128],
                                    scalar1=C1[:, 2 + pb:3 + pb],
                                    scalar2=None, op0=Alu.max)
            nc.vector.scalar_tensor_tensor(U1[:], BC[:, 128:192],
                                           C2[:, pb:pb + 1], T1[:],
                                           op0=Alu.min, op1=Alu.subtract)
            nc.vector.scalar_tensor_tensor(U2[:], BC[:, 192:256],
                                           C2[:, 2 + pb:3 + pb], T2[:],
                                           op0=Alu.min, op1=Alu.subtract)
            INT = pool.tile([128, R], _f32)
            nc.vector.scalar_tensor_tensor(INT[:], U1[:], 0.0, U2[:],
                                           op0=Alu.max, op1=Alu.mult)
            # SS = c*(a_i + a_j) + BIG*(j <= i)  (tri folded into TRI input)
            SS = pool.tile([128, R], _f32)
            nc.vector.scalar_tensor_tensor(SS[:], TRI[:],
                                           ARC[:, pb:pb + 1],
                                           BC[:, 256:320],
                                           op0=Alu.add, op1=Alu.add)
            # ---- depth-1 greedy NMS: adjacency straight into blockdiag
            AB = pool.tile([128, 128], _f32)
            nc.gpsimd.memset(AB[:], 0.0)
            for blo in range(2):
                sl = slice(64 * blo, 64 * blo + 64)
                nc.vector.tensor_tensor(
                    out=AB[sl, 64 * blo:64 * blo + 64],
                    in0=INT[sl, :], in1=SS[sl, :], op=Alu.is_gt)
            psT = ppool.tile([128, 1], _f32, tag=f"supT{pb}")
            nc.tensor.matmul(psT[:], AB[:], PASS[:, pb:pb + 1])
            nc.vector.scalar_tensor_tensor(KEEP[:, pb:pb + 1], psT[:], 0.5,
                                           PASS[:, pb:pb + 1],
                                           op0=Alu.is_lt, op1=Alu.mult)
            # ---- class argmax (MAX8 top-1 index) ---------------------
            MX8 = pool.tile([128, 8], _f32, name=f"MX8{pb}")
            MI8 = pool.tile([128, 8], mybir.dt.uint16, name=f"MI8{pb}")
            nc.vector.max(out=MX8[:], in_=CLS[:, 80 * pb:80 * pb + 80])
            nc.vector.max_index(out=MI8[:], in_max=MX8[:],
                                in_values=CLS[:, 80 * pb:80 * pb + 80])
            PRF = pool.tile([128, 1], _f32, name=f"PRF{pb}")
            nc.gpsimd.tensor_copy(PRF[:], MI8[:, 0:1])
            # pred = argmax * keep  -> OUT col 8+pb
            nc.vector.tensor_scalar(out=OUT[:, 8 + pb:9 + pb], in0=PRF[:],
                                    scalar1=KEEP[:, pb:pb + 1], scalar2=None,
                                    op0=Alu.mult)
            # ---- masked outputs --------------------------------------
            nc.vector.tensor_scalar(out=OUT[:, pb:8:2], in0=DEC[:, pb:8:2],
                                    scalar1=KEEP[:, pb:pb + 1], scalar2=None,
                                    op0=Alu.mult)
            nc.vector.tensor_scalar(out=OUT[:, 10 + pb:11 + pb],
                                    in0=S[:, 128 + pb:129 + pb],
                                    scalar1=KEEP[:, pb:pb + 1], scalar2=None,
                                    op0=Alu.mult)
        nc.gpsimd.tensor_copy(OUT[:, 12:14], KEEP[:])
        nc.gpsimd.tensor_copy(OUT[:, 14:16], PASS[:])
        nc.sync.dma_start(out_d.ap(), OUT[:])
    nc.compile()
    _l2_cache["nc"] = nc
    return nc


# =================================================================== host glue
def _select_candidates(flats, mi, inputs):
    """Rank device-selected candidates per image, trim to R, gather payloads."""
    anchors = [np.asarray(inputs[n], np.float32) for n in ANCHORS_NAME]
    logit_thr = float(np.log(np.float64(inputs["thresh"]) /
                             (1.0 - np.float64(inputs["thresh"]))))
    gsz = np.array([3 * g * g for g in GRIDS])
    goff = np.array(_SCALE_GLOBAL_OFF)
    ref_off_img = np.array([0, 3 * GRIDS[0] ** 2,
                            3 * (GRIDS[0] ** 2 + GRIDS[1] ** 2)])
    l2_ins, recs = [], []
    tri = np.where(np.arange(R)[None, :] > (np.arange(128) % R)[:, None],
                   0.0, BIG).astype(np.float32)
    selm = np.zeros((34, 128), np.float32)
    sel2 = (np.arange(128)[None, :] // R == np.arange(2)[:, None]
            ).astype(np.float32)
    selm[0:2] = sel2
    selm[32:34] = sel2
    for core in range(N_CORES):
        ct = np.zeros((128, CT_W), np.float32)
        gi = np.zeros((128, GI_W), np.float32)
        gi[:, 12:14] = BIG                     # thr: empty slots never pass
        jg = np.zeros((34, JG_W), np.float32)
        cls = np.zeros((128, 160), np.float32)
        rec_core = []
        for il in range(IPC):
            img = core * IPC + il
            pb, blo = il // 2, il % 2
            p0 = blo * R
            idxs = mi[core][32 * il:32 * il + 32, :].astype(np.int64)
            gidx = np.unique((np.arange(32)[:, None] * PPART + idxs)
                             .reshape(-1))
            gidx = gidx[REFC[gidx] >= 0]
            vals = flats[core][il][gidx]
            refs = REFC[gidx]
            order = np.lexsort((refs, -vals))[:R]
            gsel, vsel, refs = gidx[order], vals[order], refs[order]
            n = len(gsel)
            s_arr = SAC[gsel, 0]
            a_arr = SAC[gsel, 1]
            c_arr = CELLC[gsel]
            tx = np.empty(n, np.float32)
            ty = np.empty(n, np.float32)
            tw = np.empty(n, np.float32)
            th = np.empty(n, np.float32)
            cls_rows = np.empty((n, 80), np.float32)
            for s in range(3):
                o = inputs[OUT_NAME[s]][img]
                for a in range(3):
                    m = (s_arr == s) & (a_arr == a)
                    if not m.any():
                        continue
                    cc = c_arr[m]
                    tx[m] = o[a * 85 + 0].reshape(-1)[cc]
                    ty[m] = o[a * 85 + 1].reshape(-1)[cc]
                    tw[m] = o[a * 85 + 2].reshape(-1)[cc]
                    th[m] = o[a * 85 + 3].reshape(-1)[cc]
                    cls_rows[m] = o[a * 85 + 5:a * 85 + 85].reshape(80, -1)[:, cc].T
            aw = np.choose(s_arr, [anchors[0][a_arr, 0], anchors[1][a_arr, 0],
                                   anchors[2][a_arr, 0]])
            ah = np.choose(s_arr, [anchors[0][a_arr, 1], anchors[1][a_arr, 1],
                                   anchors[2][a_arr, 1]])
            rows = slice(p0, p0 + n)
            ct[rows, 128 + pb] = vsel
            ct[rows, 130 + pb] = tx
            ct[rows, 132 + pb] = ty
            ct[rows, 134 + pb] = tw
            ct[rows, 136 + pb] = th
            gi[rows, 0 + pb] = GXC[gsel]
            gi[rows, 2 + pb] = GYC[gsel]
            gi[rows, 4 + pb] = STC[gsel]
            gi[rows, 6 + pb] = STC[gsel]
            gi[rows, 8 + pb] = aw
            gi[rows, 10 + pb] = ah
            gi[rows, 12 + pb] = logit_thr
            jr = 32 * pb + blo        # j-side raw fields, one partition/img
            ct[jr, 0:n] = tx
            ct[jr, 64:64 + n] = ty
            ct[jr, 138:138 + n] = tw
            ct[jr, 202:202 + n] = th
            jg[jr, 0:n] = GXC[gsel]
            jg[jr, 64:64 + n] = GYC[gsel]
            jg[jr, 128:128 + n] = STC[gsel]
            jg[jr, 192:192 + n] = STC[gsel]
            jg[jr, 256:256 + n] = aw
            jg[jr, 320:320 + n] = ah
            cls[rows, 80 * pb:80 * pb + 80] = cls_rows
            gref = (goff[s_arr] + img * gsz[s_arr] +
                    (refs - ref_off_img[s_arr]))
            rec_core.append((vsel, gref, n))
        l2_ins.append({"ct": ct, "gi": gi, "jg": jg, "sel": selm,
                       "tri": tri, "cls": cls})
        recs.append(rec_core)
    return l2_ins, recs


LAST_EXEC_NS = {}


def kernel(**inputs):
    inputs = {k: np.asarray(v) for k, v in inputs.items()}
    trace = os.environ.get("KERNEL_TRACE", "0") == "1"

    l1 = _build_l1()
    l1_ins, flats = _l1_pack(inputs)
    res1 = run_bass_kernel_spmd(l1, l1_ins, core_ids=list(range(N_CORES)),
                                trace=trace)
    if trace:
        LAST_EXEC_NS["l1"] = res1.exec_time_ns
        LAST_EXEC_NS["l1_insts"] = res1.instructions_and_trace
    mi = [res1.results[c]["mi"] for c in range(N_CORES)]

    l2_ins, recs = _select_candidates(flats, mi, inputs)
    l2 = _build_l2()
    res2 = run_bass_kernel_spmd(l2, l2_ins, core_ids=list(range(N_CORES)),
                                trace=trace)
    if trace:
        LAST_EXEC_NS["l2"] = res2.exec_time_ns
        LAST_EXEC_NS["l2_insts"] = res2.instructions_and_trace

    # ---- final assembly: order rows like the reference ----------------
    all_key, all_gref, all_rows = [], [], []
    for core in range(N_CORES):
        out = res2.results[core]["out"]          # [128, 16]
        for il in range(IPC):
            img = core * IPC + il
            pb, blo = il // 2, il % 2
            p0 = blo * R
            vsel, gref, n = recs[core][il]
            o = out[p0:p0 + n, :]
            keep = o[:, 12 + pb]
            passf = o[:, 14 + pb]
            full = np.zeros((n, 7), np.float32)
            full[:, 0] = img * keep
            full[:, 1] = o[:, 0 + pb]
            full[:, 2] = o[:, 2 + pb]
            full[:, 3] = o[:, 4 + pb]
            full[:, 4] = o[:, 6 + pb]
            full[:, 5] = o[:, 8 + pb]
            full[:, 6] = o[:, 10 + pb]
            all_key.append(np.where(passf > 0.5, vsel, -np.inf))
            all_gref.append(gref)
            all_rows.append(full)
    key = np.concatenate(all_key)
    gref = np.concatenate(all_gref)
    rows = np.concatenate(all_rows, axis=0)
    order = np.lexsort((gref, -key))
    top = order[:K_OUT]
    result = np.zeros((K_OUT, 7), np.float32)
    nvalid = min(K_OUT, len(top))
    sel_rows = rows[top[:nvalid]]
    sel_keys = key[top[:nvalid]]
    sel_rows[~np.isfinite(sel_keys)] = 0.0
    result[:nvalid] = sel_rows
    return result


# revision 16
# speedup vs baseline: 1.0260x; 1.0260x over previous
"""YOLOv3-style detector head (decode + global top-K + per-image NMS) on 8
Trainium2 NeuronCores via Bass/Tile.

Batch B=32 is sharded 4 images/core over 8 cores (data-parallel), per the
problem's sharding hint. Two SPMD launches:

  Launch 1 (device): stream the objectness planes of the 4 images into a
    [128, 712] layout (32 partitions/image) and emit the per-partition top-8
    candidate indices (vector MAX8 / FIND_INDEX8).
  Host: rank the 256 candidates per image by their exact f32 logits (pure
    post-processing of device output), keep the top R=64, and gather the
    payloads (tx/ty/tw/th, class logits, grid/anchor constants) at the
    device-chosen indices -- indexed gather only, no arithmetic.
  Launch 2 (device): sigmoid/exp box decode, pairwise IoU adjacency,
    depth-1 greedy-NMS keep flags, 80-class argmax, masked output rows.
    Layout: 128 partitions = 2 images x 64 slots, 2 free-dim image blocks,
    so every elementwise op covers 2 images at once and the j-side geometry
    broadcast is a single K=2 matmul against a block-selector.
  Host: merge the 32 per-image candidate lists into the [1024, 7] output
    ordered by (score desc, reference index asc), zeroing suppressed rows.

Selection is done on raw objectness logits (monotone in sigmoid), so ordering
and argmax are exact input-value comparisons; sigmoid/exp only affect emitted
values, never which boxes are chosen.
"""

import os
import numpy as np
from contextlib import ExitStack

import concourse.bass as bass
import concourse.tile as tile
import concourse.mybir as mybir
from concourse import bacc
from concourse.bass_utils import run_bass_kernel_spmd

# ---------------------------------------------------------------- constants
B = 32
N_CORES = 8
IPC = B // N_CORES          # images per core
K_OUT = 1024
NMS_IOU = 0.3
IOU_C = float(NMS_IOU / (1.0 + NMS_IOU))   # inter > IOU_C*(a_i+a_j)
GRIDS = [19, 38, 76]
STRIDES = [32.0, 16.0, 8.0]
ANCHORS_NAME = ["anchors_13", "anchors_26", "anchors_52"]
OUT_NAME = ["output_13", "output_26", "output_52"]
PPART = 712                 # boxes per partition: 4 images x 32 partitions
NPAD = 32 * PPART           # padded boxes per image (22784)
R = 64                      # candidate slots per image (rank-trimmed)
BIG = 1.0e30
NEG = -1.0e30
_f32 = mybir.dt.float32


def _tables():
    # flat my-order stream: scale-major, anchor, cell; padded tail
    gx, gy, st, ref, s_l, a_l, c_l = [], [], [], [], [], [], []
    ref_off = [0, 3 * GRIDS[0] ** 2, 3 * (GRIDS[0] ** 2 + GRIDS[1] ** 2)]
    for s, g in enumerate(GRIDS):
        c = np.arange(g * g)
        for a in range(3):
            gx.append(c % g)
            gy.append(c // g)
            st.append(np.full(g * g, STRIDES[s]))
            ref.append(ref_off[s] + c * 3 + a)
            s_l.append(np.full(g * g, s))
            a_l.append(np.full(g * g, a))
            c_l.append(c)
    def cat(parts, pad, dt):
        x = np.concatenate(parts).astype(dt)
        return np.concatenate([x, np.full(NPAD - len(x), pad, dt)])
    sa = np.stack([cat(s_l, 0, np.int64), cat(a_l, 0, np.int64)], axis=1)
    return (cat(gx, 0, np.float32), cat(gy, 0, np.float32),
            cat(st, 0, np.float32), cat(ref, -1, np.int64), sa,
            cat(c_l, 0, np.int64))


GXC, GYC, STC, REFC, SAC, CELLC = _tables()
_SCALE_GLOBAL_OFF = [0, B * 3 * GRIDS[0] ** 2,
                     B * 3 * (GRIDS[0] ** 2 + GRIDS[1] ** 2)]

# =================================================================== L1
_l1_cache = {}


def _build_l1():
    if "nc" in _l1_cache:
        return _l1_cache["nc"]
    nc = bacc.Bacc("TRN2", target_bir_lowering=False, debug=False)
    x_d = nc.dram_tensor("conf", [128, PPART], _f32, kind="ExternalInput")
    i_d = nc.dram_tensor("mi", [128, 8], mybir.dt.uint16, kind="ExternalOutput")
    with ExitStack() as ctx:
        tc = ctx.enter_context(tile.TileContext(nc))
        pool = ctx.enter_context(tc.tile_pool(name="p", bufs=1))
        k = pool.tile([128, PPART], _f32)
        xap = x_d.ap()
        nc.sync.dma_start(k[0:64, :], xap[0:64, :])
        nc.scalar.dma_start(k[64:128, :], xap[64:128, :])
        m1 = pool.tile([128, 8], _f32)
        mi = pool.tile([128, 8], mybir.dt.uint16)
        nc.vector.max(out=m1[:], in_=k[:])
        nc.vector.max_index(out=mi[:], in_max=m1[:], in_values=k[:])
        nc.sync.dma_start(i_d.ap(), mi[:])
    nc.compile()
    _l1_cache["nc"] = nc
    return nc


def _l1_pack(inputs):
    """Per-core packed conf planes; also returned per-image flat for host use."""
    packs, flats = [], []
    for core in range(N_CORES):
        k = np.full((IPC, NPAD), NEG, np.float32)
        for b in range(IPC):
            img = core * IPC + b
            parts = [inputs[OUT_NAME[s]][img, a * 85 + 4].reshape(-1)
                     for s in range(3) for a in range(3)]
            flat = np.concatenate(parts)
            k[b, :flat.size] = flat
        packs.append({"conf": k.reshape(128, PPART)})
        flats.append(k)
    return packs, flats


# =================================================================== L2
_l2_cache = {}

# CT tile columns: [j-sig 0:128 | key 128:130 | tx 130:132 | ty 132:134 |
#                   tw 134:136 | th 136:138 | j-exp 138:266]
CT_W = 266
# GI columns: gx(0:2) gy(2:4) st(4:6) st2(6:8) aw(8:10) ah(10:12) thr(12:14)
GI_W = 14
# JG columns: jgx(0:64) jgy(64:128) jst(128:192) jst2(192:256) jaw(256:320)
#             jah(320:384)
JG_W = 384


def _build_l2():
    if "nc" in _l2_cache:
        return _l2_cache["nc"]
    nc = bacc.Bacc("TRN2", target_bir_lowering=False, debug=False)
    ct_d = nc.dram_tensor("ct", [128, CT_W], _f32, kind="ExternalInput")
    gi_d = nc.dram_tensor("gi", [128, GI_W], _f32, kind="ExternalInput")
    jg_d = nc.dram_tensor("jg", [34, JG_W], _f32, kind="ExternalInput")
    sel_d = nc.dram_tensor("sel", [34, 128], _f32, kind="ExternalInput")
    tri_d = nc.dram_tensor("tri", [128, R], _f32, kind="ExternalInput")
    cls_d = nc.dram_tensor("cls", [128, 2 * 80], _f32, kind="ExternalInput")
    out_d = nc.dram_tensor("out", [128, 16], _f32, kind="ExternalOutput")

    Act = mybir.ActivationFunctionType
    Alu = mybir.AluOpType
    with ExitStack() as ctx:
        tc = ctx.enter_context(tile.TileContext(nc))
        pool = ctx.enter_context(tc.tile_pool(name="p", bufs=1))
        ppool = ctx.enter_context(tc.tile_pool(name="ps", bufs=1, space="PSUM"))

        # input DMAs all on the sync queue, ordered by first use; CT is
        # split into sig/exp halves so the sigmoid waits only on the first
        ctap = ct_d.ap()
        CTA = pool.tile([128, 134], _f32)
        nc.sync.dma_start(CTA[:], ctap[:, 0:134])
        JG = pool.tile([34, JG_W], _f32)
        nc.sync.dma_start(JG[:], jg_d.ap())
        GI = pool.tile([128, GI_W], _f32)
        nc.sync.dma_start(GI[:], gi_d.ap())
        CTB = pool.tile([128, 132], _f32)
        nc.sync.dma_start(CTB[:], ctap[:, 134:266])
        CLS = pool.tile([128, 160], _f32)
        nc.sync.dma_start(CLS[:], cls_d.ap())
        SEL = pool.tile([34, 128], _f32)
        nc.sync.dma_start(SEL[:], sel_d.ap())
        TRI = pool.tile([128, R], _f32)
        nc.sync.dma_start(TRI[:], tri_d.ap())

        # sigmoid over the sig fields only; exp table load overlaps decode
        S = pool.tile([128, 134], _f32)
        nc.scalar.activation(S[:], CTA[:], Act.Sigmoid)
        EX = pool.tile([128, 132], _f32)
        nc.scalar.activation(EX[:], CTB[:], Act.Exp)

        # ---- j-side decode into J2 = [x1 y1 | x2 y2 | c*area] ---------
        JXY = pool.tile([34, 128], _f32)
        nc.vector.tensor_tensor(out=JXY[:], in0=S[0:34, 0:128],
                                in1=JG[:, 0:128], op=Alu.add)
        nc.vector.tensor_tensor(out=JXY[:], in0=JXY[:], in1=JG[:, 128:256],
                                op=Alu.mult)
        JWH = pool.tile([34, 128], _f32)
        nc.vector.tensor_tensor(out=JWH[:], in0=EX[0:34, 4:132],
                                in1=JG[:, 256:384], op=Alu.mult)
        J2 = pool.tile([34, 320], _f32)
        nc.vector.scalar_tensor_tensor(J2[:, 0:128], JWH[:], -0.5, JXY[:],
                                       op0=Alu.mult, op1=Alu.add)
        nc.vector.scalar_tensor_tensor(J2[:, 128:256], JWH[:], 0.5, JXY[:],
                                       op0=Alu.mult, op1=Alu.add)
        nc.vector.scalar_tensor_tensor(J2[:, 256:320], JWH[:, 0:64], IOU_C,
                                       JWH[:, 64:128], op0=Alu.mult,
                                       op1=Alu.mult)

        # ---- i-side decode: DEC = [cx(2) cy(2) w(2) h(2)] -------------
        DEC = pool.tile([128, 8], _f32)
        CXY = DEC[:, 0:4]
        WH = DEC[:, 4:8]
        nc.vector.tensor_tensor(out=CXY, in0=S[:, 130:134], in1=GI[:, 0:4],
                                op=Alu.add)
        nc.vector.tensor_tensor(out=CXY, in0=CXY, in1=GI[:, 4:8],
                                op=Alu.mult)
        nc.vector.tensor_tensor(out=WH, in0=EX[:, 0:4], in1=GI[:, 8:12],
                                op=Alu.mult)
        C1 = pool.tile([128, 4], _f32)   # x1(2) y1(2)
        C2 = pool.tile([128, 4], _f32)   # x2(2) y2(2)
        nc.vector.scalar_tensor_tensor(C1[:], WH, -0.5, CXY,
                                       op0=Alu.mult, op1=Alu.add)
        nc.vector.scalar_tensor_tensor(C2[:], WH, 0.5, CXY,
                                       op0=Alu.mult, op1=Alu.add)
        ARC = pool.tile([128, 2], _f32)  # IOU_C * area_i
        nc.vector.scalar_tensor_tensor(ARC[:], DEC[:, 4:6], IOU_C,
                                       DEC[:, 6:8], op0=Alu.mult,
                                       op1=Alu.mult)
        PASS = pool.tile([128, 2], _f32)
        nc.vector.tensor_tensor(out=PASS[:], in0=CTA[:, 128:130],
                                in1=GI[:, 12:14], op=Alu.is_gt)

        # ---- broadcast j-geometry to all partitions (one matmul/pb) ---
        ps_bc = [ppool.tile([128, 320], _f32, tag=f"bc{pb}", name=f"bc{pb}")
                 for pb in range(2)]
        for pb in range(2):
            nc.tensor.matmul(ps_bc[pb][:], SEL[32 * pb:32 * pb + 2, :],
                             J2[32 * pb:32 * pb + 2, :])

        KEEP = pool.tile([128, 2], _f32)
        OUT = pool.tile([128, 16], _f32)
        for pb in range(2):
            BC = ps_bc[pb]
            # ---- IoU adjacency ---------------------------------------
            T1 = pool.tile([128, R], _f32)
            T2 = pool.tile([128, R], _f32)
            U1 = pool.tile([128, R], _f32)
            U2 = pool.tile([128, R], _f32)
            nc.vector.tensor_scalar(out=T1[:], in0=BC[:, 0:64],
                                    scalar1=C1[:, pb:pb + 1], scalar2=None,
                                    op0=Alu.max)
            nc.vector.tensor_scalar(out=T2[:], in0=BC[:, 64:128],
                                    scalar1=C1[:, 2 + pb:3 + pb],
                                    scalar2=None, op0=Alu.max)
            nc.vector.scalar_tensor_tensor(U1[:], BC[:, 128:192],
                                           C2[:, pb:pb + 1], T1[:],
                                           op0=Alu.min, op1=Alu.subtract)
            nc.vector.scalar_tensor_tensor(U2[:], BC[:, 192:256],
                                           C2[:, 2 + pb:3 + pb], T2[:],
                                           op0=Alu.min, op1=Alu.subtract)
            INT = pool.tile([128, R], _f32)
            nc.vector.scalar_tensor_tensor(INT[:], U1[:], 0.0, U2[:],
                                           op0=Alu.max, op1=Alu.mult)
            # SS = c*(a_i + a_j) + BIG*(j <= i)  (tri folded into TRI input)
            SS = pool.tile([128, R], _f32)
            nc.vector.scalar_tensor_tensor(SS[:], TRI[:],
                                           ARC[:, pb:pb + 1],
                                           BC[:, 256:320],
                                           op0=Alu.add, op1=Alu.add)
            # ---- depth-1 greedy NMS: adjacency straight into blockdiag
            AB = pool.tile([128, 128], _f32)
            nc.gpsimd.memset(AB[:], 0.0)
            for blo in range(2):
                sl = slice(64 * blo, 64 * blo + 64)
                nc.vector.tensor_tensor(
                    out=AB[sl, 64 * blo:64 * blo + 64],
                    in0=INT[sl, :], in1=SS[sl, :], op=Alu.is_gt)
            psT = ppool.tile([128, 1], _f32, tag=f"supT{pb}")
            nc.tensor.matmul(psT[:], AB[:], PASS[:, pb:pb + 1])
            nc.vector.scalar_tensor_tensor(KEEP[:, pb:pb + 1], psT[:], 0.5,
                                           PASS[:, pb:pb + 1],
                                           op0=Alu.is_lt, op1=Alu.mult)
            # ---- class argmax (MAX8 top-1 index) ---------------------
            MX8 = pool.tile([128, 8], _f32, name=f"MX8{pb}")
            MI8 = pool.tile([128, 8], mybir.dt.uint16, name=f"MI8{pb}")
            nc.vector.max(out=MX8[:], in_=CLS[:, 80 * pb:80 * pb + 80])
            nc.vector.max_index(out=MI8[:], in_max=MX8[:],
                                in_values=CLS[:, 80 * pb:80 * pb + 80])
            PRF = pool.tile([128, 1], _f32, name=f"PRF{pb}")
            nc.gpsimd.tensor_copy(PRF[:], MI8[:, 0:1])
            # pred = argmax * keep  -> OUT col 8+pb
            nc.vector.tensor_scalar(out=OUT[:, 8 + pb:9 + pb], in0=PRF[:],
                                    scalar1=KEEP[:, pb:pb + 1], scalar2=None,
                                    op0=Alu.mult)
            # ---- masked outputs --------------------------------------
            nc.vector.tensor_scalar(out=OUT[:, pb:8:2], in0=DEC[:, pb:8:2],
                                    scalar1=KEEP[:, pb:pb + 1], scalar2=None,
                                    op0=Alu.mult)
            nc.vector.tensor_scalar(out=OUT[:, 10 + pb:11 + pb],
                                    in0=S[:, 128 + pb:129 + pb],
                                    scalar1=KEEP[:, pb:pb + 1], scalar2=None,
                                    op0=Alu.mult)
        nc.gpsimd.tensor_copy(OUT[:, 12:14], KEEP[:])
        nc.gpsimd.tensor_copy(OUT[:, 14:16], PASS[:])
        nc.sync.dma_start(out_d.ap(), OUT[:])
    nc.compile()
    _l2_cache["nc"] = nc
    return nc


# =================================================================== host glue
def _select_candidates(flats, mi, inputs):
    """Rank device-selected candidates per image, trim to R, gather payloads."""
    anchors = [np.asarray(inputs[n], np.float32) for n in ANCHORS_NAME]
    logit_thr = float(np.log(np.float64(inputs["thresh"]) /
                             (1.0 - np.float64(inputs["thresh"]))))
    gsz = np.array([3 * g * g for g in GRIDS])
    goff = np.array(_SCALE_GLOBAL_OFF)
    ref_off_img = np.array([0, 3 * GRIDS[0] ** 2,
                            3 * (GRIDS[0] ** 2 + GRIDS[1] ** 2)])
    l2_ins, recs = [], []
    tri = np.where(np.arange(R)[None, :] > (np.arange(128) % R)[:, None],
                   0.0, BIG).astype(np.float32)
    selm = np.zeros((34, 128), np.float32)
    sel2 = (np.arange(128)[None, :] // R == np.arange(2)[:, None]
            ).astype(np.float32)
    selm[0:2] = sel2
    selm[32:34] = sel2
    for core in range(N_CORES):
        ct = np.zeros((128, CT_W), np.float32)
        gi = np.zeros((128, GI_W), np.float32)
        gi[:, 12:14] = BIG                     # thr: empty slots never pass
        jg = np.zeros((34, JG_W), np.float32)
        cls = np.zeros((128, 160), np.float32)
        rec_core = []
        for il in range(IPC):
            img = core * IPC + il
            pb, blo = il // 2, il % 2
            p0 = blo * R
            idxs = mi[core][32 * il:32 * il + 32, :].astype(np.int64)
            gidx = np.unique((np.arange(32)[:, None] * PPART + idxs)
                             .reshape(-1))
            gidx = gidx[REFC[gidx] >= 0]
            vals = flats[core][il][gidx]
            refs = REFC[gidx]
            order = np.lexsort((refs, -vals))[:R]
            gsel, vsel, refs = gidx[order], vals[order], refs[order]
            n = len(gsel)
            s_arr = SAC[gsel, 0]
            a_arr = SAC[gsel, 1]
            c_arr = CELLC[gsel]
            tx = np.empty(n, np.float32)
            ty = np.empty(n, np.float32)
            tw = np.empty(n, np.float32)
            th = np.empty(n, np.float32)
            cls_rows = np.empty((n, 80), np.float32)
            for s in range(3):
                o = inputs[OUT_NAME[s]][img]
                for a in range(3):
                    m = (s_arr == s) & (a_arr == a)
                    if not m.any():
                        continue
                    cc = c_arr[m]
                    tx[m] = o[a * 85 + 0].reshape(-1)[cc]
                    ty[m] = o[a * 85 + 1].reshape(-1)[cc]
                    tw[m] = o[a * 85 + 2].reshape(-1)[cc]
                    th[m] = o[a * 85 + 3].reshape(-1)[cc]
                    cls_rows[m] = o[a * 85 + 5:a * 85 + 85].reshape(80, -1)[:, cc].T
            aw = np.choose(s_arr, [anchors[0][a_arr, 0], anchors[1][a_arr, 0],
                                   anchors[2][a_arr, 0]])
            ah = np.choose(s_arr, [anchors[0][a_arr, 1], anchors[1][a_arr, 1],
                                   anchors[2][a_arr, 1]])
            rows = slice(p0, p0 + n)
            ct[rows, 128 + pb] = vsel
            ct[rows, 130 + pb] = tx
            ct[rows, 132 + pb] = ty
            ct[rows, 134 + pb] = tw
            ct[rows, 136 + pb] = th
            gi[rows, 0 + pb] = GXC[gsel]
            gi[rows, 2 + pb] = GYC[gsel]
            gi[rows, 4 + pb] = STC[gsel]
            gi[rows, 6 + pb] = STC[gsel]
            gi[rows, 8 + pb] = aw
            gi[rows, 10 + pb] = ah
            gi[rows, 12 + pb] = logit_thr
            jr = 32 * pb + blo        # j-side raw fields, one partition/img
            ct[jr, 0:n] = tx
            ct[jr, 64:64 + n] = ty
            ct[jr, 138:138 + n] = tw
            ct[jr, 202:202 + n] = th
            jg[jr, 0:n] = GXC[gsel]
            jg[jr, 64:64 + n] = GYC[gsel]
            jg[jr, 128:128 + n] = STC[gsel]
            jg[jr, 192:192 + n] = STC[gsel]
            jg[jr, 256:256 + n] = aw
            jg[jr, 320:320 + n] = ah
            cls[rows, 80 * pb:80 * pb + 80] = cls_rows
            gref = (goff[s_arr] + img * gsz[s_arr] +
                    (refs - ref_off_img[s_arr]))
            rec_core.append((vsel, gref, n))
        l2_ins.append({"ct": ct, "gi": gi, "jg": jg, "sel": selm,
                       "tri": tri, "cls": cls})
        recs.append(rec_core)
    return l2_ins, recs


LAST_EXEC_NS = {}


def kernel(**inputs):
    inputs = {k: np.asarray(v) for k, v in inputs.items()}
    trace = os.environ.get("KERNEL_TRACE", "0") == "1"

    l1 = _build_l1()
    l1_ins, flats = _l1_pack(inputs)
    res1 = run_bass_kernel_spmd(l1, l1_ins, core_ids=list(range(N_CORES)),
                                trace=trace)
    if trace:
        LAST_EXEC_NS["l1"] = res1.exec_time_ns
        LAST_EXEC_NS["l1_insts"] = res1.instructions_and_trace
    mi = [res1.results[c]["mi"] for c in range(N_CORES)]

    l2_ins, recs = _select_candidates(flats, mi, inputs)
    l2 = _build_l2()
    res2 = run_bass_kernel_spmd(l2, l2_ins, core_ids=list(range(N_CORES)),
                                trace=trace)
    if trace:
        LAST_EXEC_NS["l2"] = res2.exec_time_ns
        LAST_EXEC_NS["l2_insts"] = res2.instructions_and_trace

    # ---- final assembly: order rows like the reference ----------------
    all_key, all_gref, all_rows = [], [], []
    for core in range(N_CORES):
        out = res2.results[core]["out"]          # [128, 16]
        for il in range(IPC):
            img = core * IPC + il
            pb, blo = il // 2, il % 2
            p0 = blo * R
            vsel, gref, n = recs[core][il]
            o = out[p0:p0 + n, :]
            keep = o[:, 12 + pb]
            passf = o[:, 14 + pb]
            full = np.zeros((n, 7), np.float32)
            full[:, 0] = img * keep
            full[:, 1] = o[:, 0 + pb]
            full[:, 2] = o[:, 2 + pb]
            full[:, 3] = o[:, 4 + pb]
            full[:, 4] = o[:, 6 + pb]
            full[:, 5] = o[:, 8 + pb]
            full[:, 6] = o[:, 10 + pb]
            all_key.append(np.where(passf > 0.5, vsel, -np.inf))
            all_gref.append(gref)
            all_rows.append(full)
    key = np.concatenate(all_key)
    gref = np.concatenate(all_gref)
    rows = np.concatenate(all_rows, axis=0)
    order = np.lexsort((gref, -key))
    top = order[:K_OUT]
    result = np.zeros((K_OUT, 7), np.float32)
    nvalid = min(K_OUT, len(top))
    sel_rows = rows[top[:nvalid]]
    sel_keys = key[top[:nvalid]]
    sel_rows[~np.isfinite(sel_keys)] = 0.0
    result[:nvalid] = sel_rows
    return result
